# revision 8
# baseline (speedup 1.0000x reference)
"""Barlow Twins loss on 8 trn2 NeuronCores — minimal dual-Gram Bass kernel.

Math: with A = normalize(z_a), B = normalize(z_b) (per-column, ddof=1) and
c = A.T @ B / N:

    loss = sum_d (c_dd - 1)^2 + lam * sum_{d != e} c_de^2
    sum_all c^2 = tr((A A.T)(B B.T)) / N^2     (Gram matrices are [N, N])

The host normalizes (O(N*D), free), computes the exact diagonal c_dd by
column dots, and slices/transposes/quantizes per-core inputs.  Each core
receives a [1024, 256] fp8(e4m3) slice of A and of B (d = 8p + i across
128 partitions) and computes the two partial [256, 256] Grams with 32
accumulating PE matmuls into 4 PSUM banks; Grams are separable over
column shards (Ga = sum_cores A_i A_i.T).  The host reduces the 8 bf16
partials in float64 and assembles the loss.

Device schedule: inputs stream as 4 quarter-DMAs per tensor on the two
HWDGE rings (sync = A, scalar = B) so the PE can start on the first
2-tile chunk early; the PE first runs short dummy matmuls (on garbage
SBUF, result discarded) so the HAM clock gate reaches 8/8 (2.4 GHz)
during the real stream; DVE copies each PSUM bank to SBUF (bf16) as
soon as its accumulation group stops, and each ring DMAs its Gram out
as soon as both its banks land.
"""

import numpy as np

N = 256
D = 8192
NCORES = 8
D_LOCAL = D // NCORES  # 1024
P = 128
NT = D_LOCAL // P  # 8 tiles per tensor per core
NC_IN = 4  # input chunks per tensor
TPC = NT // NC_IN  # tiles per chunk = 2
LAMBDA = 0.005

N_DUMMY_MM = 24  # x ~107ns cold = ~2.6us of PE warmup bridging to first data
DUM_N = 128

_CACHE: dict = {}


def _build_program():
    import concourse.bacc as bacc
    from concourse import mybir

    f32 = mybir.dt.float32
    bf16 = mybir.dt.bfloat16
    fp8 = mybir.dt.float8e4
    fp8o = mybir.dt.float8e5

    nc = bacc.Bacc("TRN2", target_bir_lowering=False, debug=False)

    za_t = nc.dram_tensor("za_t", [D_LOCAL, N], fp8, kind="ExternalInput").ap()
    zb_t = nc.dram_tensor("zb_t", [D_LOCAL, N], fp8, kind="ExternalInput").ap()
    ga = nc.dram_tensor("ga", [P, 2, N], fp8o, kind="ExternalOutput").ap()
    gb = nc.dram_tensor("gb", [P, 2, N], fp8o, kind="ExternalOutput").ap()

    src = {
        "a": za_t.rearrange("(p i) n -> p (i n)", i=NT),
        "b": zb_t.rearrange("(p i) n -> p (i n)", i=NT),
    }

    raw = {t: nc.alloc_sbuf_tensor(f"raw_{t}", [P, NT, N], fp8).ap() for t in "ab"}
    g_sb = {t: nc.alloc_sbuf_tensor(f"g_sb_{t}", [P, 2, N], fp8o).ap() for t in "ab"}
    dummy_sb = nc.alloc_sbuf_tensor("dummy_sb", [P, DUM_N], bf16).ap()
    gps = {
        t: [nc.alloc_psum_tensor(f"g_ps_{t}{m}", [P, N], f32).ap() for m in range(2)]
        for t in "ab"
    }
    dummy_ps = nc.alloc_psum_tensor("dummy_ps", [P, DUM_N], f32).ap()

    sem = {}
    for t in "ab":
        for q in range(NC_IN):
            sem[f"d{t}{q}"] = nc.alloc_semaphore(f"d{t}{q}")
    for name in ("mma", "mmb", "cpa", "cpb", "dga", "dgb"):
        sem[name] = nc.alloc_semaphore(name)
    mms = {"a": sem["mma"], "b": sem["mmb"]}
    cps = {"a": sem["cpa"], "b": sem["cpb"]}

    CH = TPC * N  # flat elems per chunk per partition row

    with nc.Block() as block:

        @block.sync
        def _(sync):
            fa = raw["a"].rearrange("p i n -> p (i n)")
            for q in range(NC_IN):
                nc.sync.dma_start(
                    fa[:, q * CH : (q + 1) * CH], src["a"][:, q * CH : (q + 1) * CH]
                ).then_inc(sem[f"da{q}"], 16)
            nc.sync.wait_ge(sem["cpa"], 2)
            nc.sync.dma_start(ga, g_sb["a"][:]).then_inc(sem["dga"], 16)
            nc.sync.wait_ge(sem["dga"], 16)

        @block.scalar
        def _(scalar):
            fb = raw["b"].rearrange("p i n -> p (i n)")
            for q in range(NC_IN):
                nc.scalar.dma_start(
                    fb[:, q * CH : (q + 1) * CH], src["b"][:, q * CH : (q + 1) * CH]
                ).then_inc(sem[f"db{q}"], 16)
            # A-bank casts on ACT, in parallel with DVE's B-bank casts
            for m in range(2):
                nc.scalar.wait_ge(mms["a"], m + 1)
                nc.scalar.copy(g_sb["a"][:, m, :], gps["a"][m][:]).then_inc(
                    sem["cpa"], 1)
            nc.scalar.wait_ge(sem["cpb"], 2)
            nc.scalar.dma_start(gb, g_sb["b"][:]).then_inc(sem["dgb"], 16)
            nc.scalar.wait_ge(sem["dgb"], 16)

        @block.vector
        def _(vector):
            for m in range(2):
                nc.vector.wait_ge(mms["b"], m + 1)
                nc.vector.tensor_copy(g_sb["b"][:, m, :], gps["b"][m][:]).then_inc(
                    sem["cpb"], 1)

        @block.tensor
        def _(tensor):
            from bass_rust import MatmulPerfMode

            # warmup on garbage SBUF (output discarded) — no data dependency
            for _i in range(N_DUMMY_MM):
                nc.tensor.matmul(
                    dummy_ps[:], lhsT=dummy_sb[:], rhs=dummy_sb[:],
                    start=True, stop=True, skip_group_check=True,
                )
            # chunk order tracks DMA arrival: a0, b0, a1, b1, ...
            # DoubleRow fp8: one matmul covers a 2-tile plane pair (K=256)
            for q in range(NC_IN):
                for t in "ab":
                    nc.tensor.wait_ge(sem[f"d{t}{q}"], 16)
                    first = q == 0
                    last = q == NC_IN - 1
                    ts = slice(q * TPC, (q + 1) * TPC)
                    for m in range(2):
                        ins = nc.tensor.matmul(
                            gps[t][m][:],
                            lhsT=raw[t][:, ts, m * P : (m + 1) * P],
                            rhs=raw[t][:, ts, :], start=first, stop=last,
                            perf_mode=MatmulPerfMode.DoubleRow,
                        )
                        if last:
                            ins.then_inc(mms[t], 1)

    nc.compile()
    return nc


def _get_program():
    if "nc" not in _CACHE:
        _CACHE["nc"] = _build_program()
    return _CACHE["nc"]


LAST_RESULT = None


def kernel(z_a: np.ndarray, z_b: np.ndarray) -> np.ndarray:
    global LAST_RESULT
    import ml_dtypes

    from concourse.bass_utils import run_bass_kernel_spmd

    z_a = np.asarray(z_a, dtype=np.float32)
    z_b = np.asarray(z_b, dtype=np.float32)
    assert z_a.shape == (N, D) and z_b.shape == (N, D)

    nc = _get_program()

    # host: exact normalization (ddof=1) in float64
    def norm(z):
        z = z.astype(np.float64)
        mu = z.mean(axis=0)
        sd = z.std(axis=0, ddof=1)
        return (z - mu) / sd

    A = norm(z_a)
    B = norm(z_b)
    cdd = np.einsum("nd,nd->d", A, B) / N  # exact diagonal of c

    f8 = ml_dtypes.float8_e4m3fn
    in_maps = []
    for c in range(NCORES):
        sl = slice(c * D_LOCAL, (c + 1) * D_LOCAL)
        in_maps.append(
            {
                "za_t": np.ascontiguousarray(A[:, sl].T.astype(f8)),
                "zb_t": np.ascontiguousarray(B[:, sl].T.astype(f8)),
            }
        )

    res = run_bass_kernel_spmd(nc, in_maps, core_ids=list(range(NCORES)))
    LAST_RESULT = res

    Ga = np.zeros((P, 2, N), dtype=np.float64)
    Gb = np.zeros((P, 2, N), dtype=np.float64)
    for c in range(NCORES):
        out = res.results[c]
        Ga += out["ga"].astype(np.float64)
        Gb += out["gb"].astype(np.float64)
    # [p, m, n] -> row u = m*128 + p
    Ga = Ga.transpose(1, 0, 2).reshape(N, N)
    Gb = Gb.transpose(1, 0, 2).reshape(N, N)

    sum_c2 = float((Ga * Gb).sum()) / (N * N)  # sum over ALL (d, e) of c^2
    loss = (
        LAMBDA * (sum_c2 - float((cdd * cdd).sum()))
        + float(((cdd - 1.0) ** 2).sum())
    )
    return np.float32(loss)


if __name__ == "__main__":
    rng = np.random.default_rng(0)
    za = rng.standard_normal((N, D), dtype=np.float32)
    zb = rng.standard_normal((N, D), dtype=np.float32)
    out = kernel(z_a=za, z_b=zb)
    print("kernel output:", out)


# revision 11
# speedup vs baseline: 1.0571x; 1.0571x over previous
"""Barlow Twins loss on 8 trn2 NeuronCores — minimal dual-Gram Bass kernel.

Math: with A = normalize(z_a), B = normalize(z_b) (per-column, ddof=1) and
c = A.T @ B / N:

    loss = sum_d (c_dd - 1)^2 + lam * sum_{d != e} c_de^2
    sum_all c^2 = tr((A A.T)(B B.T)) / N^2     (Gram matrices are [N, N])

The host normalizes (O(N*D), free), computes the exact diagonal c_dd by
column dots, and slices/transposes/quantizes per-core inputs.  Each core
receives a [1024, 256] fp8(e4m3) slice of A and of B (d = 8p + i across
128 partitions) and computes the two partial [256, 256] Grams with 32
accumulating PE matmuls into 4 PSUM banks; Grams are separable over
column shards (Ga = sum_cores A_i A_i.T).  The host reduces the 8 bf16
partials in float64 and assembles the loss.

Device schedule: inputs stream as 4 quarter-DMAs per tensor on the two
HWDGE rings (sync = A, scalar = B) so the PE can start on the first
2-tile chunk early; the PE first runs short dummy matmuls (on garbage
SBUF, result discarded) so the HAM clock gate reaches 8/8 (2.4 GHz)
during the real stream; DVE copies each PSUM bank to SBUF (bf16) as
soon as its accumulation group stops, and each ring DMAs its Gram out
as soon as both its banks land.
"""

import numpy as np

N = 256
D = 8192
NCORES = 8
D_LOCAL = D // NCORES  # 1024
P = 128
NT = D_LOCAL // P  # 8 tiles per tensor per core
NC_IN = 4  # input chunks per tensor
TPC = NT // NC_IN  # tiles per chunk = 2
LAMBDA = 0.005

N_DUMMY_MM = 24  # x ~107ns cold = ~2.6us of PE warmup bridging to first data
DUM_N = 128

_CACHE: dict = {}


def _build_program():
    import concourse.bacc as bacc
    from concourse import mybir

    f32 = mybir.dt.float32
    bf16 = mybir.dt.bfloat16
    fp8 = mybir.dt.float8e4
    fp8o = mybir.dt.float8e5

    nc = bacc.Bacc("TRN2", target_bir_lowering=False, debug=False)

    za_t = nc.dram_tensor("za_t", [D_LOCAL, N], fp8, kind="ExternalInput").ap()
    zb_t = nc.dram_tensor("zb_t", [D_LOCAL, N], fp8, kind="ExternalInput").ap()
    ga = nc.dram_tensor("ga", [P, 2, N], fp8o, kind="ExternalOutput").ap()
    gb = nc.dram_tensor("gb", [P, 2, N], fp8o, kind="ExternalOutput").ap()

    src = {
        "a": za_t.rearrange("(p i) n -> p (i n)", i=NT),
        "b": zb_t.rearrange("(p i) n -> p (i n)", i=NT),
    }

    raw = {t: nc.alloc_sbuf_tensor(f"raw_{t}", [P, NT, N], fp8).ap() for t in "ab"}
    g_sb = {t: nc.alloc_sbuf_tensor(f"g_sb_{t}", [P, 2, N], fp8o).ap() for t in "ab"}
    dummy_sb = nc.alloc_sbuf_tensor("dummy_sb", [P, DUM_N], bf16).ap()
    gps = {
        t: [nc.alloc_psum_tensor(f"g_ps_{t}{m}", [P, N], f32).ap() for m in range(2)]
        for t in "ab"
    }
    dummy_ps = nc.alloc_psum_tensor("dummy_ps", [P, DUM_N], f32).ap()

    sem = {}
    for t in "ab":
        for q in range(NC_IN):
            sem[f"d{t}{q}"] = nc.alloc_semaphore(f"d{t}{q}")
    for name in ("mma", "mmb", "cpa", "cpb", "dga", "dgb"):
        sem[name] = nc.alloc_semaphore(name)
    mms = {"a": sem["mma"], "b": sem["mmb"]}
    cps = {"a": sem["cpa"], "b": sem["cpb"]}

    CH = TPC * N  # flat elems per chunk per partition row

    with nc.Block() as block:

        @block.sync
        def _(sync):
            fa = raw["a"].rearrange("p i n -> p (i n)")
            for q in range(NC_IN):
                nc.sync.dma_start(
                    fa[:, q * CH : (q + 1) * CH], src["a"][:, q * CH : (q + 1) * CH]
                ).then_inc(sem[f"da{q}"], 16)
            # per-bank output DMAs: the first arms the ring while the second
            # bank is still casting, hiding the ring-start latency
            for m in range(2):
                nc.sync.wait_ge(sem["cpa"], m + 1)
                nc.sync.dma_start(ga[:, m, :], g_sb["a"][:, m, :]).then_inc(
                    sem["dga"], 16)
            nc.sync.wait_ge(sem["dga"], 32)

        @block.scalar
        def _(scalar):
            fb = raw["b"].rearrange("p i n -> p (i n)")
            for q in range(NC_IN):
                nc.scalar.dma_start(
                    fb[:, q * CH : (q + 1) * CH], src["b"][:, q * CH : (q + 1) * CH]
                ).then_inc(sem[f"db{q}"], 16)
            # A-bank casts on ACT, in parallel with DVE's B-bank casts
            for m in range(2):
                nc.scalar.wait_ge(mms["a"], m + 1)
                nc.scalar.copy(g_sb["a"][:, m, :], gps["a"][m][:]).then_inc(
                    sem["cpa"], 1)
            for m in range(2):
                nc.scalar.wait_ge(sem["cpb"], m + 1)
                nc.scalar.dma_start(gb[:, m, :], g_sb["b"][:, m, :]).then_inc(
                    sem["dgb"], 16)
            nc.scalar.wait_ge(sem["dgb"], 32)

        @block.vector
        def _(vector):
            for m in range(2):
                nc.vector.wait_ge(mms["b"], m + 1)
                nc.vector.tensor_copy(g_sb["b"][:, m, :], gps["b"][m][:]).then_inc(
                    sem["cpb"], 1)

        @block.tensor
        def _(tensor):
            # warmup on garbage SBUF (output discarded) — no data dependency
            for _i in range(N_DUMMY_MM):
                nc.tensor.matmul(
                    dummy_ps[:], lhsT=dummy_sb[:], rhs=dummy_sb[:],
                    start=True, stop=True, skip_group_check=True,
                )
            # chunk order tracks DMA arrival: a0, b0, a1, b1, ...
            for q in range(NC_IN):
                for t in "ab":
                    nc.tensor.wait_ge(sem[f"d{t}{q}"], 16)
                    for i in range(q * TPC, (q + 1) * TPC):
                        first = i == 0
                        last = i == NT - 1
                        for m in range(2):
                            ins = nc.tensor.matmul(
                                gps[t][m][:],
                                lhsT=raw[t][:, i, m * P : (m + 1) * P],
                                rhs=raw[t][:, i, :], start=first, stop=last,
                            )
                            if last:
                                ins.then_inc(mms[t], 1)

    nc.compile()
    return nc


def _get_program():
    if "nc" not in _CACHE:
        _CACHE["nc"] = _build_program()
    return _CACHE["nc"]


LAST_RESULT = None


def kernel(z_a: np.ndarray, z_b: np.ndarray) -> np.ndarray:
    global LAST_RESULT
    import ml_dtypes

    from concourse.bass_utils import run_bass_kernel_spmd

    z_a = np.asarray(z_a, dtype=np.float32)
    z_b = np.asarray(z_b, dtype=np.float32)
    assert z_a.shape == (N, D) and z_b.shape == (N, D)

    nc = _get_program()

    # host: exact normalization (ddof=1) in float64
    def norm(z):
        z = z.astype(np.float64)
        mu = z.mean(axis=0)
        sd = z.std(axis=0, ddof=1)
        return (z - mu) / sd

    A = norm(z_a)
    B = norm(z_b)
    cdd = np.einsum("nd,nd->d", A, B) / N  # exact diagonal of c

    f8 = ml_dtypes.float8_e4m3fn
    in_maps = []
    for c in range(NCORES):
        sl = slice(c * D_LOCAL, (c + 1) * D_LOCAL)
        in_maps.append(
            {
                "za_t": np.ascontiguousarray(A[:, sl].T.astype(f8)),
                "zb_t": np.ascontiguousarray(B[:, sl].T.astype(f8)),
            }
        )

    res = run_bass_kernel_spmd(nc, in_maps, core_ids=list(range(NCORES)))
    LAST_RESULT = res

    Ga = np.zeros((P, 2, N), dtype=np.float64)
    Gb = np.zeros((P, 2, N), dtype=np.float64)
    for c in range(NCORES):
        out = res.results[c]
        Ga += out["ga"].astype(np.float64)
        Gb += out["gb"].astype(np.float64)
    # [p, m, n] -> row u = m*128 + p
    Ga = Ga.transpose(1, 0, 2).reshape(N, N)
    Gb = Gb.transpose(1, 0, 2).reshape(N, N)

    sum_c2 = float((Ga * Gb).sum()) / (N * N)  # sum over ALL (d, e) of c^2
    loss = (
        LAMBDA * (sum_c2 - float((cdd * cdd).sum()))
        + float(((cdd - 1.0) ** 2).sum())
    )
    return np.float32(loss)


if __name__ == "__main__":
    rng = np.random.default_rng(0)
    za = rng.standard_normal((N, D), dtype=np.float32)
    zb = rng.standard_normal((N, D), dtype=np.float32)
    out = kernel(z_a=za, z_b=zb)
    print("kernel output:", out)
